# revision 6
# baseline (speedup 1.0000x reference)
"""KAN layer (LayerNorm -> per-bin Bernstein spline -> reduce over input dim)
as a Bass/Tile kernel for 8 trn2 NeuronCores.

Sharding: data-parallel over batch (8 rows of x per core), poly_matrix
replicated.

Memory-regime design (the gather of per-(batch,input,bin) coefficient blocks
dominates: 64*512 blocks of (4,512) floats = 268MB of traffic):

* R is stored in fp8-e4m3, halving gather traffic vs bf16.  Plain fp8
  rounding of this uniform-distributed tensor gives ~2.5e-2 max rel err
  (> the 2e-2 gate), so two refinements recover accuracy (~1.36e-2):
  1. LayerNorm clips ~33% of inputs to the boundary bins 0/99.  Those bins'
     coefficients are kept exact in bf16, preloaded in SBUF (4MB), and their
     contribution is computed by a dense m=8 matmul path (no gather).
     Clipped (b,i) gather offsets are remapped to row 0 (row-buffer hits)
     and their basis weights zeroed.
  2. Interior bins use *joint* k-rounding: the 4 coefficients of a bin only
     ever appear in the combination sum_k B_k(t)*c_k with Bernstein weights,
     so the 16 up/down rounding patterns per (i,g,o) are searched for the
     one minimizing E_t[(sum_k B_k(t) eps_k)^2].

* Per-sample contraction on the PE: for each (b, i-chunk): indirect-gather
  G fp8 [128, 2048] (one 2KB row per (b,i)), then 4 matmuls
  acc[1,512] += bvT[128,1].T @ G[:, 512k:512k+512] (bf16 lhsT, fp8 rhs).
* Boundary path: 32 matmuls bacc[8,512] += bv0/99T[128,8].T @ C[128,512]
  over SBUF-resident exact tables; combined with acc on the DVE.
"""

import hashlib
import os
import tempfile

import numpy as np
import ml_dtypes

import concourse.bass as bass
import concourse.mybir as mybir
import concourse.tile as tile
from concourse import bacc
from concourse.bass_utils import run_bass_kernel_spmd
from concourse.masks import make_identity

B = 64          # total batch
D_IN = 512
D_OUT = 512
DEG = 3
GRID = 100
GRID_EPS = 1e-6
LN_EPS = 1e-5
N_CORES = 8
BPC = B // N_CORES          # batch rows per core (8)
NROWS = D_IN * GRID         # 51200 gatherable rows
ROW = (DEG + 1) * D_OUT     # 2048 elements per row (k-major, o-minor)
NCH = D_IN // 128           # 4 i-chunks of 128
NK = DEG + 1

F32 = mybir.dt.float32
F8 = mybir.dt.float8e4
BF16 = mybir.dt.bfloat16
I32 = mybir.dt.int32
AX = mybir.AxisListType
OP = mybir.AluOpType
AF = mybir.ActivationFunctionType

NP_F8 = ml_dtypes.float8_e4m3
NP_BF16 = ml_dtypes.bfloat16

_CACHE = {}


def _build_nc(Mconst, apply_affine, repeat=1):
    """Build + compile the per-core Bass program.

    Mconst: 4x4 python floats of basis_matrix (power->Bernstein).
    repeat: if >1, wrap the gather+contract phase in a HW loop with this
    trip count (timing builds only; repeat=1 emits no loop).
    """
    nc = bacc.Bacc("TRN2", target_bir_lowering=False, debug=False)

    x8 = nc.declare_dram_parameter("x8", [BPC, D_IN], F32, isOutput=False)
    R = nc.declare_dram_parameter("R", [NROWS, ROW], F8, isOutput=False)
    # boundary tables: [D_IN, 2*ROW] bf16; cols = [bin0: k-major o | bin99: ...]
    C = nc.declare_dram_parameter("C", [D_IN, 2 * ROW], BF16, isOutput=False)
    if apply_affine:
        w8 = nc.declare_dram_parameter("w8", [BPC, D_IN], F32, isOutput=False)
        b8 = nc.declare_dram_parameter("b8", [BPC, D_IN], F32, isOutput=False)
    y8 = nc.declare_dram_parameter("y8", [BPC, D_OUT], F32, isOutput=True)

    NB = 13 * BPC   # stacked rows: offs(8) + bvint(32) + bv0(32) + bv99(32)

    with tile.TileContext(nc) as tc:
        with (
            tc.tile_pool(name="const", bufs=1) as cp,
            tc.tile_pool(name="work", bufs=1) as wp,
            tc.tile_pool(name="gpool", bufs=8) as gp,
            tc.tile_pool(name="outp", bufs=2) as op_,
            tc.tile_pool(name="yp", bufs=4) as yp,
            tc.tile_pool(name="ptr", bufs=2, space="PSUM") as ptr,
            tc.tile_pool(name="pacc", bufs=4, space="PSUM") as pacc,
            tc.tile_pool(name="pbnd", bufs=1, space="PSUM") as pbnd,
        ):
            ident = cp.tile([128, 128], F32, tag="ident")
            make_identity(nc, ident[:])

            # row-base offsets i*GRID, as f32 (exact ints < 2^24)
            iotaI = cp.tile([BPC, D_IN], I32, tag="iotaI")
            nc.gpsimd.iota(iotaI[:], pattern=[[GRID, D_IN]], base=0,
                           channel_multiplier=0)
            iotaF = cp.tile([BPC, D_IN], F32, tag="iotaF")
            nc.vector.tensor_copy(iotaF[:], iotaI[:])

            # boundary tables into SBUF, one [128, 2*ROW] tile per i-chunk
            Csb = []
            for c in range(NCH):
                ct = cp.tile([128, 2 * ROW], BF16, tag=f"C{c}")
                nc.sync.dma_start(ct[:], C[c * 128:(c + 1) * 128, :])
                Csb.append(ct)

            x = wp.tile([BPC, D_IN], F32, tag="x")
            nc.sync.dma_start(x[:], x8[:])

            # ---- LayerNorm (two-pass, matching jnp.mean/jnp.var) ----
            sumx = wp.tile([BPC, 1], F32, tag="sumx")
            nc.vector.tensor_reduce(sumx[:], x[:], axis=AX.X, op=OP.add)
            mean = wp.tile([BPC, 1], F32, tag="mean")
            nc.vector.tensor_scalar_mul(mean[:], sumx[:], 1.0 / D_IN)
            xc = wp.tile([BPC, D_IN], F32, tag="xc")
            nc.vector.tensor_scalar(xc[:], x[:], mean[:, :1], None, OP.subtract)
            sq = wp.tile([BPC, D_IN], F32, tag="sq")
            nc.scalar.square(sq[:], xc[:])
            v = wp.tile([BPC, 1], F32, tag="v")
            nc.vector.tensor_reduce(v[:], sq[:], axis=AX.X, op=OP.add)
            nc.vector.tensor_scalar(v[:], v[:], 1.0 / D_IN, LN_EPS, OP.mult, OP.add)
            # rstd = rsqrt(v) via sqrt + reciprocal + one Newton step
            s = wp.tile([BPC, 1], F32, tag="s")
            nc.scalar.sqrt(s[:], v[:])
            r0 = wp.tile([BPC, 1], F32, tag="r0")
            nc.vector.reciprocal(r0[:], s[:])
            r2 = wp.tile([BPC, 1], F32, tag="r2")
            nc.vector.tensor_tensor(out=r2[:], in0=r0[:], in1=r0[:], op=OP.mult)
            nc.vector.tensor_tensor(out=r2[:], in0=r2[:], in1=v[:], op=OP.mult)
            nc.vector.tensor_scalar(r2[:], r2[:], -0.5, 1.5, OP.mult, OP.add)
            rstd = wp.tile([BPC, 1], F32, tag="rstd")
            nc.vector.tensor_tensor(out=rstd[:], in0=r0[:], in1=r2[:], op=OP.mult)

            xn = wp.tile([BPC, D_IN], F32, tag="xn")
            nc.vector.tensor_scalar(xn[:], xc[:], rstd[:, :1], None, OP.mult)
            if apply_affine:
                wt = wp.tile([BPC, D_IN], F32, tag="wt")
                bt = wp.tile([BPC, D_IN], F32, tag="bt")
                nc.sync.dma_start(wt[:], w8[:])
                nc.sync.dma_start(bt[:], b8[:])
                nc.vector.tensor_tensor(out=xn[:], in0=xn[:], in1=wt[:], op=OP.mult)
                nc.vector.tensor_tensor(out=xn[:], in0=xn[:], in1=bt[:], op=OP.add)

            # clip, map to [0, GRID)
            cl = wp.tile([BPC, D_IN], F32, tag="cl")
            nc.vector.tensor_scalar(cl[:], xn[:], -1.0 + GRID_EPS, 1.0 - GRID_EPS,
                                    OP.max, OP.min)
            u = wp.tile([BPC, D_IN], F32, tag="u")
            nc.vector.tensor_scalar(u[:], cl[:], 1.0, 0.5, OP.add, OP.mult)
            nc.vector.tensor_scalar(u[:], u[:], float(GRID), None, OP.mult)

            # floor(u) robust to either int-conversion rounding mode
            i1 = wp.tile([BPC, D_IN], I32, tag="i1")
            nc.vector.tensor_copy(i1[:], u[:])
            f1 = wp.tile([BPC, D_IN], F32, tag="f1")
            nc.vector.tensor_copy(f1[:], i1[:])
            gt = wp.tile([BPC, D_IN], F32, tag="gt")
            nc.vector.tensor_tensor(out=gt[:], in0=f1[:], in1=u[:], op=OP.is_gt)
            flr = wp.tile([BPC, D_IN], F32, tag="flr")
            nc.vector.tensor_tensor(out=flr[:], in0=f1[:], in1=gt[:], op=OP.subtract)
            t = wp.tile([BPC, D_IN], F32, tag="t")
            nc.vector.tensor_tensor(out=t[:], in0=u[:], in1=flr[:], op=OP.subtract)

            # boundary masks and interior selector
            m0 = wp.tile([BPC, D_IN], F32, tag="m0")
            nc.vector.tensor_scalar(m0[:], flr[:], 0.0, None, OP.is_equal)
            m99 = wp.tile([BPC, D_IN], F32, tag="m99")
            nc.vector.tensor_scalar(m99[:], flr[:], float(GRID - 1), None,
                                    OP.is_equal)
            sel = wp.tile([BPC, D_IN], F32, tag="sel")
            nc.vector.tensor_tensor(out=sel[:], in0=m0[:], in1=m99[:], op=OP.add)
            nc.vector.tensor_scalar(sel[:], sel[:], -1.0, 1.0, OP.mult, OP.add)

            # ---- Bernstein basis via Horner ----
            bv = []
            for k in range(NK):
                m3, m2, m1, m0c = (Mconst[3][k], Mconst[2][k],
                                   Mconst[1][k], Mconst[0][k])
                h = wp.tile([BPC, D_IN], F32, tag=f"bv{k}")
                nc.scalar.activation(h[:], t[:], AF.Copy, bias=m2, scale=m3)
                nc.vector.tensor_tensor(out=h[:], in0=h[:], in1=t[:], op=OP.mult)
                nc.scalar.activation(h[:], h[:], AF.Copy, bias=m1, scale=1.0)
                nc.vector.tensor_tensor(out=h[:], in0=h[:], in1=t[:], op=OP.mult)
                nc.scalar.activation(h[:], h[:], AF.Copy, bias=m0c, scale=1.0)
                bv.append(h)

            # ---- masked basis bands (each [8, 512] at partition base 0) ----
            offsF = wp.tile([BPC, D_IN], F32, tag="offsF")
            # offs = (flr + i*GRID) * sel  (clipped entries -> row 0)
            nc.vector.tensor_tensor(out=offsF[:], in0=flr[:], in1=iotaF[:],
                                    op=OP.add)
            nc.vector.tensor_tensor(out=offsF[:], in0=offsF[:], in1=sel[:],
                                    op=OP.mult)
            bands = []       # 12 tiles in bvT column order
            for mask_t in (sel, m0, m99):
                for k in range(NK):
                    bb = wp.tile([BPC, D_IN], F32, tag=f"band{len(bands)}")
                    nc.vector.tensor_tensor(out=bb[:], in0=bv[k][:],
                                            in1=mask_t[:], op=OP.mult)
                    bands.append(bb)

            # ---- transpose to [128 i, ...] per chunk ----
            offsT = []
            bvT = []
            for c in range(NCH):
                sl = slice(c * 128, (c + 1) * 128)
                pt = ptr.tile([128, BPC], F32, tag="ptr")
                nc.tensor.transpose(pt[:], offsF[:, sl], ident[:BPC, :BPC])
                ot = cp.tile([128, BPC], I32, tag=f"offsT{c}")
                nc.vector.tensor_copy(ot[:], pt[:])
                offsT.append(ot)
                bt_ = cp.tile([128, 12 * BPC], BF16, tag=f"bvT{c}")
                for j, bb in enumerate(bands):
                    pb = ptr.tile([128, BPC], F32, tag="ptr")
                    nc.tensor.transpose(pb[:], bb[:, sl], ident[:BPC, :BPC])
                    nc.vector.tensor_copy(bt_[:, j * BPC:(j + 1) * BPC], pb[:])
                bvT.append(bt_)

            # bvT col layout: [bvint k*8+b | bv0 32+k*8+b | bv99 64+k*8+b]

            def phase_body(_i=None):
                ysb = op_.tile([BPC, D_OUT], F32, tag="ysb")
                for b in range(BPC):
                    acc = pacc.tile([1, D_OUT], F32, tag="acc")
                    for c in range(NCH):
                        G = gp.tile([128, ROW], F8, tag="G")
                        nc.gpsimd.indirect_dma_start(
                            out=G[:],
                            out_offset=None,
                            in_=R[:],
                            in_offset=bass.IndirectOffsetOnAxis(
                                ap=offsT[c][:, b:b + 1], axis=0),
                        )
                        for k in range(NK):
                            nc.tensor.matmul(
                                acc[:],
                                lhsT=bvT[c][:, k * BPC + b:k * BPC + b + 1],
                                rhs=G[:, k * D_OUT:(k + 1) * D_OUT],
                                start=(c == 0 and k == 0),
                                stop=(c == NCH - 1 and k == NK - 1),
                            )
                    # per-sample accumulator -> its row of ysb.  Engines
                    # cannot write partition base b, so: ACT copy to a
                    # base-0 row tile, then SBUF->SBUF DMA into the row.
                    yrow = yp.tile([1, D_OUT], F32, tag="yrow")
                    nc.scalar.activation(yrow[:], acc[:], AF.Copy,
                                         bias=0.0, scale=1.0)
                    nc.sync.dma_start(ysb[b:b + 1, :], yrow[:])

                # boundary bins: dense m=8 matmuls over exact bf16 tables
                bacc = pbnd.tile([BPC, D_OUT], F32, tag="bacc")
                n_mm = 2 * NCH * NK
                j = 0
                for side in range(2):
                    for c in range(NCH):
                        for k in range(NK):
                            col0 = (1 + side) * 4 * BPC + k * BPC
                            nc.tensor.matmul(
                                bacc[:],
                                lhsT=bvT[c][:, col0:col0 + BPC],
                                rhs=Csb[c][:, side * ROW + k * D_OUT:
                                           side * ROW + (k + 1) * D_OUT],
                                start=(j == 0),
                                stop=(j == n_mm - 1),
                            )
                            j += 1
                bout = op_.tile([BPC, D_OUT], F32, tag="bout")
                nc.vector.tensor_copy(bout[:], bacc[:])
                nc.vector.tensor_tensor(out=ysb[:], in0=ysb[:], in1=bout[:],
                                        op=OP.add)
                nc.sync.dma_start(y8[:], ysb[:])

            if repeat == 1:
                phase_body()
            else:
                with tc.For_i(0, repeat) as _i:
                    phase_body(_i)

    nc.compile()
    return nc


def _joint_round_fp8(pm, Mmat):
    """Quantize poly_matrix (D_IN, D_OUT, GRID, NK) to fp8-e4m3, choosing per
    (i,o,g) the up/down rounding pattern over k minimizing
    E_t[(sum_k B_k(t) eps_k)^2], t ~ U(0,1).  Returns fp8 array, same shape."""
    # Bernstein moment matrix E_t[B B^T]
    ts = (np.arange(201, dtype=np.float64) + 0.5) / 201
    Bt = (ts[:, None] ** np.arange(NK)) @ Mmat.astype(np.float64)
    Mm = (Bt[:, :, None] * Bt[:, None, :]).mean(0).astype(np.float32)

    grid8 = np.arange(256, dtype=np.uint8).view(NP_F8).astype(np.float32)
    grid8 = np.sort(np.unique(grid8[np.isfinite(grid8)]))

    out = np.empty(pm.shape, dtype=NP_F8)
    pairs = [(j, k) for j in range(NK) for k in range(j + 1, NK)]
    for i0 in range(0, pm.shape[0], 32):
        blk = np.ascontiguousarray(pm[i0:i0 + 32], dtype=np.float32)
        li = np.searchsorted(grid8, blk, side='right') - 1
        lo = grid8[np.clip(li, 0, len(grid8) - 1)]
        hi = grid8[np.clip(li + 1, 0, len(grid8) - 1)]
        r = blk - lo                      # >= 0
        d = hi - lo
        # score(s) = const - 2 s.(d*(Mm r)) + s.(d^2 diag) + cross terms
        Mr = np.einsum('ab,...b->...a', Mm, r)
        A = d * (np.einsum('aa->a', Mm) * d - 2.0 * Mr)        # (..., K)
        Bx = {(j, k): 2.0 * Mm[j, k] * d[..., j] * d[..., k] for j, k in pairs}
        best_score = np.zeros(blk.shape[:-1], np.float32)
        best_mask = np.zeros(blk.shape[:-1], np.int8)
        first = True
        for mask in range(16):
            sc = np.zeros(blk.shape[:-1], np.float32)
            for k in range(NK):
                if (mask >> k) & 1:
                    sc += A[..., k]
            for (j, k) in pairs:
                if ((mask >> j) & 1) and ((mask >> k) & 1):
                    sc += Bx[(j, k)]
            if first:
                best_score, best_mask, first = sc, best_mask, False
            else:
                upd = sc < best_score
                best_score = np.where(upd, sc, best_score)
                best_mask = np.where(upd, np.int8(mask), best_mask)
        q = lo.copy()
        for k in range(NK):
            take_hi = ((best_mask >> k) & 1).astype(bool)
            q[..., k] = np.where(take_hi, hi[..., k], lo[..., k])
        out[i0:i0 + 32] = q.astype(NP_F8)
    return out


def _prep_tables(poly_matrix, basis_matrix):
    """R fp8 [NROWS, ROW] (row=(i,g), col=k-major o) + C bf16 [D_IN, 2*ROW]."""
    pm = np.asarray(poly_matrix, np.float32)
    key = hashlib.sha1(pm.tobytes()).hexdigest()[:16]
    cache = os.path.join(tempfile.gettempdir(), f"kan_fp8_{key}.npz")
    if os.path.exists(cache):
        z = np.load(cache)
        return z["R"].view(NP_F8), z["C"].view(NP_BF16)
    q = _joint_round_fp8(pm, np.asarray(basis_matrix, np.float32))
    # R[i, g, k, o] <- q[i, o, g, k]
    Rq = np.ascontiguousarray(np.transpose(q, (0, 2, 3, 1))).reshape(NROWS, ROW)
    # boundary tables exact in bf16: C[i, side, k, o]
    Cb = np.empty((D_IN, 2, NK, D_OUT), dtype=NP_BF16)
    Cb[:, 0] = np.transpose(pm[:, :, 0, :], (0, 2, 1)).astype(NP_BF16)
    Cb[:, 1] = np.transpose(pm[:, :, GRID - 1, :], (0, 2, 1)).astype(NP_BF16)
    Cb = Cb.reshape(D_IN, 2 * ROW)
    np.savez(cache, R=Rq.view(np.uint8), C=Cb.view(np.uint16))
    return Rq, Cb


def get_compiled(basis_matrix, ln_weight, ln_bias, repeat=1):
    apply_affine = not (np.all(ln_weight == 1.0) and np.all(ln_bias == 0.0))
    Mkey = np.asarray(basis_matrix, np.float32).tobytes()
    key = (Mkey, apply_affine, repeat)
    if key not in _CACHE:
        Mconst = [[float(basis_matrix[j, k]) for k in range(NK)]
                  for j in range(NK)]
        _CACHE[key] = _build_nc(Mconst, apply_affine, repeat)
    return _CACHE[key], apply_affine


def make_in_maps(x, poly_matrix, ln_weight, ln_bias, apply_affine,
                 basis_matrix):
    Rq, Cb = _prep_tables(poly_matrix, basis_matrix)
    x = np.asarray(x, np.float32)
    maps = []
    for c in range(N_CORES):
        m = {"x8": np.ascontiguousarray(x[c * BPC:(c + 1) * BPC]),
             "R": Rq, "C": Cb}
        if apply_affine:
            m["w8"] = np.ascontiguousarray(
                np.broadcast_to(np.asarray(ln_weight, np.float32), (BPC, D_IN)))
            m["b8"] = np.ascontiguousarray(
                np.broadcast_to(np.asarray(ln_bias, np.float32), (BPC, D_IN)))
        maps.append(m)
    return maps


def kernel(x, poly_matrix, ln_weight, ln_bias, basis_matrix):
    nc, apply_affine = get_compiled(basis_matrix, ln_weight, ln_bias)
    in_maps = make_in_maps(x, poly_matrix, ln_weight, ln_bias, apply_affine,
                           basis_matrix)
    res = run_bass_kernel_spmd(nc, in_maps, core_ids=list(range(N_CORES)))
    y = np.concatenate([res.results[c]["y8"] for c in range(N_CORES)], axis=0)
    return y.astype(np.float32)


# revision 14
# speedup vs baseline: 1097.8640x; 1097.8640x over previous
"""KAN layer (LayerNorm -> per-bin Bernstein spline -> reduce over input dim)
as a Bass/Tile kernel for 8 trn2 NeuronCores.

Sharding: data-parallel over batch (8 rows of x per core), poly_matrix
replicated.

Memory-regime design (the gather of per-(batch,input,bin) coefficient blocks
dominates: 64*512 blocks of (4,512) floats = 268MB of traffic):

* R is stored in fp8-e4m3, halving gather traffic vs bf16.  Plain fp8
  rounding of this uniform-distributed tensor gives ~2.5e-2 max rel err
  (> the 2e-2 gate), so two refinements recover accuracy (~1.36e-2):
  1. LayerNorm clips ~33% of inputs to the boundary bins 0/99.  Those bins'
     coefficients are kept exact in bf16, preloaded in SBUF (4MB), and their
     contribution is computed by a dense m=8 matmul path (no gather).
     Clipped (b,i) gather offsets are remapped to row 0 (row-buffer hits)
     and their basis weights zeroed.
  2. Interior bins use *joint* k-rounding: the 4 coefficients of a bin only
     ever appear in the combination sum_k B_k(t)*c_k with Bernstein weights,
     so the 16 up/down rounding patterns per (i,g,o) are searched for the
     one minimizing E_t[(sum_k B_k(t) eps_k)^2].

* Per-sample contraction on the PE: for each (b, i-chunk): indirect-gather
  G fp8 [128, 2048] (one 2KB row per (b,i)), then 4 matmuls
  acc[1,512] += bvT[128,1].T @ G[:, 512k:512k+512] (bf16 lhsT, fp8 rhs).
* Boundary path: 32 matmuls bacc[8,512] += bv0/99T[128,8].T @ C[128,512]
  over SBUF-resident exact tables; combined with acc on the DVE.
"""

import hashlib
import os
import tempfile

import numpy as np
import ml_dtypes

import concourse.bass as bass
import concourse.mybir as mybir
import concourse.tile as tile
from concourse import bacc
from concourse.bass_utils import run_bass_kernel_spmd
from concourse.masks import make_identity

B = 64          # total batch
D_IN = 512
D_OUT = 512
DEG = 3
GRID = 100
GRID_EPS = 1e-6
LN_EPS = 1e-5
N_CORES = 8
BPC = B // N_CORES          # batch rows per core (8)
NROWS = D_IN * GRID         # 51200 gatherable rows
ROW = (DEG + 1) * D_OUT     # 2048 elements per row (k-major, o-minor)
NCH = D_IN // 128           # 4 i-chunks of 128
NK = DEG + 1

F32 = mybir.dt.float32
F8 = mybir.dt.float8e4
BF16 = mybir.dt.bfloat16
I32 = mybir.dt.int32
AX = mybir.AxisListType
OP = mybir.AluOpType
AF = mybir.ActivationFunctionType

NP_F8 = ml_dtypes.float8_e4m3
NP_BF16 = ml_dtypes.bfloat16

_CACHE = {}


def _build_nc(Mconst, apply_affine, repeat=1, variant="full"):
    """Build + compile the per-core Bass program.

    Mconst: 4x4 python floats of basis_matrix (power->Bernstein).
    repeat: if >1, wrap the gather+contract phase in a HW loop with this
    trip count (timing builds only; repeat=1 emits no loop).
    """
    nc = bacc.Bacc("TRN2", target_bir_lowering=False, debug=False)

    x8 = nc.declare_dram_parameter("x8", [BPC, D_IN], F32, isOutput=False)
    R = nc.declare_dram_parameter("R", [NROWS, ROW], F8, isOutput=False)
    # boundary tables: [D_IN, 2*ROW] bf16; cols = [bin0: k-major o | bin99: ...]
    C = nc.declare_dram_parameter("C", [D_IN, 2 * ROW], BF16, isOutput=False)
    R2 = None
    if variant == "nomm4k":
        R2 = nc.declare_dram_parameter("R2", [NROWS // 2, 2 * ROW], F8,
                                       isOutput=False)
    if apply_affine:
        w8 = nc.declare_dram_parameter("w8", [BPC, D_IN], F32, isOutput=False)
        b8 = nc.declare_dram_parameter("b8", [BPC, D_IN], F32, isOutput=False)
    y8 = nc.declare_dram_parameter("y8", [BPC, D_OUT], F32, isOutput=True)

    NB = 13 * BPC
    do_dma = variant in ("full", "nomm", "nocmb", "bg", "cg",
                         "nommskip", "nomm4k")
    do_mm = variant in ("full", "nodma", "nocmb", "bg", "cg")
    do_cmb = variant in ("full", "nodma", "nomm", "bg", "cg",
                         "nommskip", "nomm4k")
    batched_gather = variant in ("bg",)
    colgrp = variant in ("full", "nodma", "cg")
    skip_clip = variant in ("nommskip",)


    with tile.TileContext(nc) as tc:
        with (
            tc.tile_pool(name="const", bufs=1) as cp,
            tc.tile_pool(name="work", bufs=1) as wp,
            tc.tile_pool(name="gpool", bufs=4) as gp,
            tc.tile_pool(name="outp", bufs=2) as op_,
            tc.tile_pool(name="yp", bufs=4) as yp,
            tc.tile_pool(name="oc", bufs=2) as op2,
            tc.tile_pool(name="ptr", bufs=2, space="PSUM") as ptr,
            tc.tile_pool(name="pacc", bufs=2, space="PSUM") as pacc,
            tc.tile_pool(name="pbnd", bufs=1, space="PSUM") as pbnd,
        ):
            ident = cp.tile([128, 128], F32, tag="ident")
            make_identity(nc, ident[:])

            # row-base offsets i*GRID, as f32 (exact ints < 2^24)
            iotaI = cp.tile([BPC, D_IN], I32, tag="iotaI")
            nc.gpsimd.iota(iotaI[:], pattern=[[GRID, D_IN]], base=0,
                           channel_multiplier=0)
            iotaF = cp.tile([BPC, D_IN], F32, tag="iotaF")
            nc.vector.tensor_copy(iotaF[:], iotaI[:])

            # boundary tables into SBUF, one [128, 2*ROW] tile per i-chunk
            Csb = []
            for c in range(NCH):
                ct = cp.tile([128, 2 * ROW], BF16, tag=f"C{c}")
                nc.sync.dma_start(ct[:], C[c * 128:(c + 1) * 128, :])
                Csb.append(ct)

            x = wp.tile([BPC, D_IN], F32, tag="x")
            nc.sync.dma_start(x[:], x8[:])

            # ---- LayerNorm (two-pass, matching jnp.mean/jnp.var) ----
            sumx = wp.tile([BPC, 1], F32, tag="sumx")
            nc.vector.tensor_reduce(sumx[:], x[:], axis=AX.X, op=OP.add)
            mean = wp.tile([BPC, 1], F32, tag="mean")
            nc.vector.tensor_scalar_mul(mean[:], sumx[:], 1.0 / D_IN)
            xc = wp.tile([BPC, D_IN], F32, tag="xc")
            nc.vector.tensor_scalar(xc[:], x[:], mean[:, :1], None, OP.subtract)
            sq = wp.tile([BPC, D_IN], F32, tag="sq")
            nc.scalar.square(sq[:], xc[:])
            v = wp.tile([BPC, 1], F32, tag="v")
            nc.vector.tensor_reduce(v[:], sq[:], axis=AX.X, op=OP.add)
            nc.vector.tensor_scalar(v[:], v[:], 1.0 / D_IN, LN_EPS, OP.mult, OP.add)
            # rstd = rsqrt(v) via sqrt + reciprocal + one Newton step
            s = wp.tile([BPC, 1], F32, tag="s")
            nc.scalar.sqrt(s[:], v[:])
            r0 = wp.tile([BPC, 1], F32, tag="r0")
            nc.vector.reciprocal(r0[:], s[:])
            r2 = wp.tile([BPC, 1], F32, tag="r2")
            nc.vector.tensor_tensor(out=r2[:], in0=r0[:], in1=r0[:], op=OP.mult)
            nc.vector.tensor_tensor(out=r2[:], in0=r2[:], in1=v[:], op=OP.mult)
            nc.vector.tensor_scalar(r2[:], r2[:], -0.5, 1.5, OP.mult, OP.add)
            rstd = wp.tile([BPC, 1], F32, tag="rstd")
            nc.vector.tensor_tensor(out=rstd[:], in0=r0[:], in1=r2[:], op=OP.mult)

            xn = wp.tile([BPC, D_IN], F32, tag="xn")
            nc.vector.tensor_scalar(xn[:], xc[:], rstd[:, :1], None, OP.mult)
            if apply_affine:
                wt = wp.tile([BPC, D_IN], F32, tag="wt")
                bt = wp.tile([BPC, D_IN], F32, tag="bt")
                nc.sync.dma_start(wt[:], w8[:])
                nc.sync.dma_start(bt[:], b8[:])
                nc.vector.tensor_tensor(out=xn[:], in0=xn[:], in1=wt[:], op=OP.mult)
                nc.vector.tensor_tensor(out=xn[:], in0=xn[:], in1=bt[:], op=OP.add)

            # clip, map to [0, GRID)
            cl = wp.tile([BPC, D_IN], F32, tag="cl")
            nc.vector.tensor_scalar(cl[:], xn[:], -1.0 + GRID_EPS, 1.0 - GRID_EPS,
                                    OP.max, OP.min)
            u = wp.tile([BPC, D_IN], F32, tag="u")
            nc.vector.tensor_scalar(u[:], cl[:], 1.0, 0.5, OP.add, OP.mult)
            nc.vector.tensor_scalar(u[:], u[:], float(GRID), None, OP.mult)

            # floor(u) robust to either int-conversion rounding mode
            i1 = wp.tile([BPC, D_IN], I32, tag="i1")
            nc.vector.tensor_copy(i1[:], u[:])
            f1 = wp.tile([BPC, D_IN], F32, tag="f1")
            nc.vector.tensor_copy(f1[:], i1[:])
            gt = wp.tile([BPC, D_IN], F32, tag="gt")
            nc.vector.tensor_tensor(out=gt[:], in0=f1[:], in1=u[:], op=OP.is_gt)
            flr = wp.tile([BPC, D_IN], F32, tag="flr")
            nc.vector.tensor_tensor(out=flr[:], in0=f1[:], in1=gt[:], op=OP.subtract)
            t = wp.tile([BPC, D_IN], F32, tag="t")
            nc.vector.tensor_tensor(out=t[:], in0=u[:], in1=flr[:], op=OP.subtract)

            # boundary masks and interior selector
            m0 = wp.tile([BPC, D_IN], F32, tag="m0")
            nc.vector.tensor_scalar(m0[:], flr[:], 0.0, None, OP.is_equal)
            m99 = wp.tile([BPC, D_IN], F32, tag="m99")
            nc.vector.tensor_scalar(m99[:], flr[:], float(GRID - 1), None,
                                    OP.is_equal)
            sel = wp.tile([BPC, D_IN], F32, tag="sel")
            nc.vector.tensor_tensor(out=sel[:], in0=m0[:], in1=m99[:], op=OP.add)
            nc.vector.tensor_scalar(sel[:], sel[:], -1.0, 1.0, OP.mult, OP.add)

            # ---- Bernstein basis via Horner ----
            bv = []
            for k in range(NK):
                m3, m2, m1, m0c = (Mconst[3][k], Mconst[2][k],
                                   Mconst[1][k], Mconst[0][k])
                h = wp.tile([BPC, D_IN], F32, tag=f"bv{k}")
                nc.scalar.activation(h[:], t[:], AF.Copy, bias=m2, scale=m3)
                nc.vector.tensor_tensor(out=h[:], in0=h[:], in1=t[:], op=OP.mult)
                nc.scalar.activation(h[:], h[:], AF.Copy, bias=m1, scale=1.0)
                nc.vector.tensor_tensor(out=h[:], in0=h[:], in1=t[:], op=OP.mult)
                nc.scalar.activation(h[:], h[:], AF.Copy, bias=m0c, scale=1.0)
                bv.append(h)

            # ---- masked basis bands (each [8, 512] at partition base 0) ----
            offsF = wp.tile([BPC, D_IN], F32, tag="offsF")
            # offs = (flr + i*GRID) * sel  (clipped entries -> row 0)
            nc.vector.tensor_tensor(out=offsF[:], in0=flr[:], in1=iotaF[:],
                                    op=OP.add)
            nc.vector.tensor_tensor(out=offsF[:], in0=offsF[:], in1=sel[:],
                                    op=OP.mult)
            # OOB variant of offsets: clipped entries -> row NROWS, which the
            # gather's bounds check silently skips (saves 1/3 of the traffic)
            selN = wp.tile([BPC, D_IN], F32, tag="selN")
            nc.vector.tensor_scalar(selN[:], sel[:], -float(NROWS),
                                    float(NROWS), OP.mult, OP.add)
            offsFX = wp.tile([BPC, D_IN], F32, tag="offsFX")
            nc.vector.tensor_tensor(out=offsFX[:], in0=offsF[:], in1=selN[:],
                                    op=OP.add)
            bands = []       # 12 tiles in bvT column order
            for mask_t in (sel, m0, m99):
                for k in range(NK):
                    bb = wp.tile([BPC, D_IN], F32, tag=f"band{len(bands)}")
                    nc.vector.tensor_tensor(out=bb[:], in0=bv[k][:],
                                            in1=mask_t[:], op=OP.mult)
                    bands.append(bb)

            # ---- transpose to [128 i, ...] per chunk ----
            offsT = []
            bvT = []
            for c in range(NCH):
                sl = slice(c * 128, (c + 1) * 128)
                pt = ptr.tile([128, BPC], F32, tag="ptr")
                osrc = offsFX if skip_clip else offsF
                nc.tensor.transpose(pt[:], osrc[:, sl], ident[:BPC, :BPC])
                ot = cp.tile([128, BPC], I32, tag=f"offsT{c}")
                nc.vector.tensor_copy(ot[:], pt[:])
                offsT.append(ot)
                bt_ = cp.tile([128, 12 * BPC], BF16, tag=f"bvT{c}")
                for j, bb in enumerate(bands):
                    pb = ptr.tile([128, BPC], F32, tag="ptr")
                    nc.tensor.transpose(pb[:], bb[:, sl], ident[:BPC, :BPC])
                    nc.vector.tensor_copy(bt_[:, j * BPC:(j + 1) * BPC], pb[:])
                bvT.append(bt_)

            # bvT col layout: [bvint k*8+b | bv0 32+k*8+b | bv99 64+k*8+b]

            GW = NCH * ROW          # per-sample gather width (4 chunks)
            Gc = None
            if not do_dma and do_mm:
                Gc = cp.tile([128, ROW], F8, tag="Gc")
                nc.sync.dma_start(Gc[:], R[0:128, :])

            # combined transposed offsets: col = b*NCH + c so each
            # per-sample slice is contiguous (DMA requirement)
            offsAll = cp.tile([128, NCH * BPC], I32, tag="offsAll")
            for c in range(NCH):
                nc.vector.tensor_copy(
                    offsAll[:, c:c + (BPC - 1) * NCH + 1:NCH], offsT[c][:])

            # explicit gather destination tiles, pre-zeroed so OOB-skipped
            # lanes hold benign (finite) data on the first pass
            NGT = 8
            Gts = []
            if do_dma and not batched_gather and variant != "nomm4k":
                for j in range(NGT):
                    gt_ = gp.tile([128, ROW], F8, tag=f"Gt{j}")
                    nc.vector.memset(gt_[:], 0.0)
                    Gts.append(gt_)
            offsH = None
            if variant == "nomm4k":
                offsH = cp.tile([128, NCH * BPC], I32, tag="offsH")
                ph = ptr.tile([128, NCH * BPC], F32, tag="ph")
                nc.vector.tensor_copy(ph[:], offsAll[:])
                nc.vector.tensor_scalar(ph[:], ph[:], 0.4999, None, OP.mult)
                nc.vector.tensor_copy(offsH[:], ph[:])

            def phase_body(_i=None):
                ysb = op_.tile([BPC, D_OUT], F32, tag="ysb")
                acc4a = acc4b = yca = ycb = None
                if do_mm and colgrp:
                    # per-sample accumulators live at 32-aligned partitions so
                    # four matmul chains run concurrently in the 4 PE column
                    # groups (tile_position=(0, 32j)).
                    acc4a = pacc.tile([128, D_OUT], F32, tag="acc4a")
                    acc4b = pacc.tile([128, D_OUT], F32, tag="acc4b")
                    yca = op2.tile([128, D_OUT], F32, tag="yca")
                    ycb = op2.tile([128, D_OUT], F32, tag="ycb")
                for b in range(BPC):
                    Gs = []          # (rhs_tile, col_base) per chunk
                    if do_dma and batched_gather:
                        G = gp.tile([128, GW], F8, tag="G")
                        nc.gpsimd.indirect_dma_start(
                            out=G[:],
                            out_offset=None,
                            in_=R[:],
                            in_offset=bass.IndirectOffsetOnAxis(
                                ap=offsAll[:, b * NCH:(b + 1) * NCH],
                                axis=0),
                        )
                        Gs = [(G, c * ROW) for c in range(NCH)]
                    elif do_dma and variant == "nomm4k":
                        for c in range(NCH // 2):
                            Gi = gp.tile([128, 2 * ROW], F8, tag="Gi")
                            nc.gpsimd.indirect_dma_start(
                                out=Gi[:],
                                out_offset=None,
                                in_=R2[:],
                                in_offset=bass.IndirectOffsetOnAxis(
                                    ap=offsH[:, b * NCH + c:b * NCH + c + 1],
                                    axis=0),
                            )
                            Gs = Gs + [(Gi, 0), (Gi, ROW)]
                    elif do_dma:
                        for c in range(NCH):
                            Gi = Gts[(b * NCH + c) % NGT]
                            nc.gpsimd.indirect_dma_start(
                                out=Gi[:],
                                out_offset=None,
                                in_=R[:],
                                in_offset=bass.IndirectOffsetOnAxis(
                                    ap=offsAll[:, b * NCH + c:b * NCH + c + 1],
                                    axis=0),
                                bounds_check=NROWS - 1 if skip_clip else None,
                                oob_is_err=not skip_clip,
                            )
                            Gs = Gs + [(Gi, 0)]
                    else:
                        Gs = [(Gc, 0) for _ in range(NCH)]
                    if not do_mm:
                        continue
                    if colgrp:
                        acct = acc4a if b < 4 else acc4b
                        p0 = 32 * (b % 4)
                        out_ap = acct[p0:p0 + 1, :]
                        tp = (0, p0)
                    else:
                        acc = pacc.tile([1, D_OUT], F32, tag="acc")
                        out_ap = acc[:]
                        tp = None
                    for c in range(NCH):
                        Gt, cbase = Gs[c]
                        for k in range(NK):
                            nc.tensor.matmul(
                                out_ap,
                                lhsT=bvT[c][:, k * BPC + b:k * BPC + b + 1],
                                rhs=Gt[:, cbase + k * D_OUT:
                                       cbase + (k + 1) * D_OUT],
                                start=(c == 0 and k == 0),
                                stop=(c == NCH - 1 and k == NK - 1),
                                tile_position=tp,
                            )
                    if do_cmb and colgrp:
                        yct = yca if b < 4 else ycb
                        nc.scalar.activation(yct[p0:p0 + 1, :], out_ap,
                                             AF.Copy, bias=0.0, scale=1.0)
                    elif do_cmb:
                        yrow = yp.tile([1, D_OUT], F32, tag="yrow")
                        nc.scalar.activation(yrow[:], out_ap, AF.Copy,
                                             bias=0.0, scale=1.0)
                        nc.sync.dma_start(ysb[b:b + 1, :], yrow[:])

                # boundary bins: dense m=8 matmuls over exact bf16 tables
                if do_mm:
                    bacc = pbnd.tile([BPC, D_OUT], F32, tag="bacc")
                    n_mm = 2 * NCH * NK
                    j = 0
                    for side in range(2):
                        for c in range(NCH):
                            for k in range(NK):
                                col0 = (1 + side) * 4 * BPC + k * BPC
                                nc.tensor.matmul(
                                    bacc[:],
                                    lhsT=bvT[c][:, col0:col0 + BPC],
                                    rhs=Csb[c][:, side * ROW + k * D_OUT:
                                               side * ROW + (k + 1) * D_OUT],
                                    start=(j == 0),
                                    stop=(j == n_mm - 1),
                                    tile_position=(0, 0) if colgrp else None,
                                )
                                j += 1
                if do_mm and do_cmb:
                    if colgrp:
                        nc.sync.dma_start(ysb[0:4, :], yca[0:97:32, :])
                        nc.sync.dma_start(ysb[4:8, :], ycb[0:97:32, :])
                    bout = op_.tile([BPC, D_OUT], F32, tag="bout")
                    nc.vector.tensor_copy(bout[:], bacc[:])
                    nc.vector.tensor_tensor(out=ysb[:], in0=ysb[:],
                                            in1=bout[:], op=OP.add)
                elif do_mm:
                    nc.vector.tensor_copy(ysb[:], bacc[:])
                else:
                    nc.vector.tensor_copy(ysb[:], Csb[0][:BPC, 0:D_OUT])
                nc.sync.dma_start(y8[:], ysb[:])

            if repeat == 1:
                phase_body()
            else:
                with tc.For_i(0, repeat) as _i:
                    phase_body(_i)

    nc.compile()
    return nc


def _joint_round_fp8(pm, Mmat):
    """Quantize poly_matrix (D_IN, D_OUT, GRID, NK) to fp8-e4m3, choosing per
    (i,o,g) the up/down rounding pattern over k minimizing
    E_t[(sum_k B_k(t) eps_k)^2], t ~ U(0,1).  Returns fp8 array, same shape."""
    # Bernstein moment matrix E_t[B B^T]
    ts = (np.arange(201, dtype=np.float64) + 0.5) / 201
    Bt = (ts[:, None] ** np.arange(NK)) @ Mmat.astype(np.float64)
    Mm = (Bt[:, :, None] * Bt[:, None, :]).mean(0).astype(np.float32)

    grid8 = np.arange(256, dtype=np.uint8).view(NP_F8).astype(np.float32)
    grid8 = np.sort(np.unique(grid8[np.isfinite(grid8)]))

    out = np.empty(pm.shape, dtype=NP_F8)
    pairs = [(j, k) for j in range(NK) for k in range(j + 1, NK)]
    for i0 in range(0, pm.shape[0], 32):
        blk = np.ascontiguousarray(pm[i0:i0 + 32], dtype=np.float32)
        li = np.searchsorted(grid8, blk, side='right') - 1
        lo = grid8[np.clip(li, 0, len(grid8) - 1)]
        hi = grid8[np.clip(li + 1, 0, len(grid8) - 1)]
        r = blk - lo                      # >= 0
        d = hi - lo
        # score(s) = const - 2 s.(d*(Mm r)) + s.(d^2 diag) + cross terms
        Mr = np.einsum('ab,...b->...a', Mm, r)
        A = d * (np.einsum('aa->a', Mm) * d - 2.0 * Mr)        # (..., K)
        Bx = {(j, k): 2.0 * Mm[j, k] * d[..., j] * d[..., k] for j, k in pairs}
        best_score = np.zeros(blk.shape[:-1], np.float32)
        best_mask = np.zeros(blk.shape[:-1], np.int8)
        first = True
        for mask in range(16):
            sc = np.zeros(blk.shape[:-1], np.float32)
            for k in range(NK):
                if (mask >> k) & 1:
                    sc += A[..., k]
            for (j, k) in pairs:
                if ((mask >> j) & 1) and ((mask >> k) & 1):
                    sc += Bx[(j, k)]
            if first:
                best_score, best_mask, first = sc, best_mask, False
            else:
                upd = sc < best_score
                best_score = np.where(upd, sc, best_score)
                best_mask = np.where(upd, np.int8(mask), best_mask)
        q = lo.copy()
        for k in range(NK):
            take_hi = ((best_mask >> k) & 1).astype(bool)
            q[..., k] = np.where(take_hi, hi[..., k], lo[..., k])
        out[i0:i0 + 32] = q.astype(NP_F8)
    return out


def _prep_tables(poly_matrix, basis_matrix):
    """R fp8 [NROWS, ROW] (row=(i,g), col=k-major o) + C bf16 [D_IN, 2*ROW]."""
    pm = np.asarray(poly_matrix, np.float32)
    key = hashlib.sha1(pm.tobytes()).hexdigest()[:16]
    cache = os.path.join(tempfile.gettempdir(), f"kan_fp8_{key}.npz")
    if os.path.exists(cache):
        z = np.load(cache)
        return z["R"].view(NP_F8), z["C"].view(NP_BF16)
    q = _joint_round_fp8(pm, np.asarray(basis_matrix, np.float32))
    # R[i, g, k, o] <- q[i, o, g, k]
    Rq = np.ascontiguousarray(np.transpose(q, (0, 2, 3, 1))).reshape(NROWS, ROW)
    # boundary tables exact in bf16: C[i, side, k, o]
    Cb = np.empty((D_IN, 2, NK, D_OUT), dtype=NP_BF16)
    Cb[:, 0] = np.transpose(pm[:, :, 0, :], (0, 2, 1)).astype(NP_BF16)
    Cb[:, 1] = np.transpose(pm[:, :, GRID - 1, :], (0, 2, 1)).astype(NP_BF16)
    Cb = Cb.reshape(D_IN, 2 * ROW)
    np.savez(cache, R=Rq.view(np.uint8), C=Cb.view(np.uint16))
    return Rq, Cb


def get_compiled(basis_matrix, ln_weight, ln_bias, repeat=1, variant="full"):
    apply_affine = not (np.all(ln_weight == 1.0) and np.all(ln_bias == 0.0))
    Mkey = np.asarray(basis_matrix, np.float32).tobytes()
    key = (Mkey, apply_affine, repeat, variant)
    if key not in _CACHE:
        Mconst = [[float(basis_matrix[j, k]) for k in range(NK)]
                  for j in range(NK)]
        _CACHE[key] = _build_nc(Mconst, apply_affine, repeat, variant)
    return _CACHE[key], apply_affine


def make_in_maps(x, poly_matrix, ln_weight, ln_bias, apply_affine,
                 basis_matrix):
    Rq, Cb = _prep_tables(poly_matrix, basis_matrix)
    x = np.asarray(x, np.float32)
    maps = []
    for c in range(N_CORES):
        m = {"x8": np.ascontiguousarray(x[c * BPC:(c + 1) * BPC]),
             "R": Rq, "C": Cb, "R2": Rq.reshape(NROWS // 2, 2 * ROW)}
        if apply_affine:
            m["w8"] = np.ascontiguousarray(
                np.broadcast_to(np.asarray(ln_weight, np.float32), (BPC, D_IN)))
            m["b8"] = np.ascontiguousarray(
                np.broadcast_to(np.asarray(ln_bias, np.float32), (BPC, D_IN)))
        maps.append(m)
    return maps


def kernel(x, poly_matrix, ln_weight, ln_bias, basis_matrix):
    nc, apply_affine = get_compiled(basis_matrix, ln_weight, ln_bias)
    in_maps = make_in_maps(x, poly_matrix, ln_weight, ln_bias, apply_affine,
                           basis_matrix)
    res = run_bass_kernel_spmd(nc, in_maps, core_ids=list(range(N_CORES)))
    y = np.concatenate([res.results[c]["y8"] for c in range(N_CORES)], axis=0)
    return y.astype(np.float32)
